# revision 5
# baseline (speedup 1.0000x reference)
"""Trainium2 Bass kernel for nn_CPCircuitLayer (embedding_lookup), v4.

Math: out[b, n] = dot(A[b, idx_s[n]], Bm[b, idx_h[n]]) = M_b[idx_s[n], idx_h[n]]
where M_b = A_b @ Bm_b^T is a [S, H] table, A = X W_seq^T, Bm^T = W_hid X.

Sharding (8 cores, no collectives): core c handles batch b = c//4 and the
h-slice q = c%4 (columns [256q, 256q+256)). It builds only its M slice
[1024, 256] laid out as [128 partitions, 2048]: partition p = s%128,
offset o = (s//128)*256 + h%256. Each output n is routed to the core
owning its table element.

Per-core device pipeline:
  1. PE matmuls (bf16 in, f32 psum): A^T [32,1024], B^T [32,256], then
     M = A @ B^T -> sbuf bf16 [128, 2048].
  2. local_scatter cascade: pass k serves the rank-k user of every table
     element (host-prepped per-partition dst slots, in key order). Pass 0
     scans the table; pass k>=1 scans pass (k-1)'s dst buffer, which is
     exactly the compacted list of elements with >= k users. Scan lengths
     shrink geometrically, so 12 passes (multiplicity <= 12) cost barely
     more than 3. The packed dst buffer IS the output: one DMA to DRAM
     (split across queues), host unpermutes.
  3. Outputs whose element overflowed a dst cap or has multiplicity > 12
     (never happens for uniform random indices) are computed on host.
"""

import numpy as np
import ml_dtypes
from contextlib import ExitStack

import concourse.bass as bass
import concourse.mybir as mybir
import concourse.tile as tile
from concourse import bacc

B, S, H, R = 2, 1024, 1024, 32
N = S * H
NCORES = 8
HQ = H // 4           # h-columns per core
E = 2048              # table elements per partition (1024*256/128)
DSTS = [1440, 672, 240, 72, 24, 16, 8, 4, 4]  # per-pass dst caps
PASSES = len(DSTS)
SCANS = [E] + DSTS[:-1]                # pass k scans pass k-1's dst
DOFF = np.cumsum([0] + DSTS).tolist()  # dst offsets in packed output
IOFF = np.cumsum([0] + SCANS).tolist()  # idx offsets in packed si input
ODW = DOFF[-1]        # packed output width (2832)
SIW = IOFF[-1]        # packed int16 input width (4872)

F32 = mybir.dt.float32
BF16 = mybir.dt.bfloat16
I16 = mybir.dt.int16


def _build(reps: int = 1, compile: bool = True):
    nc = bacc.Bacc()
    xst = nc.declare_dram_parameter("xst", [512, H], BF16, False)
    xs = nc.declare_dram_parameter("xs", [S, 512], BF16, False)
    wseq_t = nc.declare_dram_parameter("wseq_t", [H, R], BF16, False)
    whid_t = nc.declare_dram_parameter("whid_t", [S, R], BF16, False)
    si_all = nc.declare_dram_parameter("si_all", [128, SIW], I16, False)
    od = nc.declare_dram_parameter("od", [128, ODW], BF16, True)

    with tile.TileContext(nc) as tc, ExitStack() as ctx:
        base = ctx.enter_context(tc.tile_pool(name="base", bufs=1))
        psA = ctx.enter_context(tc.tile_pool(name="psA", bufs=2, space="PSUM"))
        psM = ctx.enter_context(tc.tile_pool(name="psM", bufs=2, space="PSUM"))
        ab = ctx.enter_context(tc.tile_pool(name="ab", bufs=2))
        mp = ctx.enter_context(tc.tile_pool(name="mp", bufs=2))
        dsp = ctx.enter_context(tc.tile_pool(name="dsp", bufs=2))

        # --- one-time loads ----------------------------------------------
        xt_sb = base.tile([128, 8, 512], BF16)    # X^T[h,s-half]
        xs_sb = base.tile([128, 8, 512], BF16)    # X[:, h-half]
        ws_sb = base.tile([128, 8, R], BF16)      # W_seq^T rows (h-major)
        wh_sb = base.tile([128, 8, R], BF16)      # W_hid^T rows (s-major)
        si_sb = base.tile([128, SIW], I16)        # cascade idx streams

        for k in range(8):
            nc.sync.dma_start_transpose(
                out=xt_sb[:, k, :], in_=xst[:, 128 * k:128 * (k + 1)]
            )
        nc.sync.dma_start(
            out=xs_sb[:],
            in_=bass.AP(tensor=xs[:].tensor, offset=0,
                        ap=[[512, 128], [128 * 512, 8], [1, 512]]),
        )
        nc.sync.dma_start(
            out=ws_sb[:],
            in_=bass.AP(tensor=wseq_t[:].tensor, offset=0,
                        ap=[[R, 128], [128 * R, 8], [1, R]]),
        )
        nc.sync.dma_start(
            out=wh_sb[:],
            in_=bass.AP(tensor=whid_t[:].tensor, offset=0,
                        ap=[[R, 128], [128 * R, 8], [1, R]]),
        )
        nc.sync.dma_start(out=si_sb[:], in_=si_all[:])

        for _ in range(reps):
            _body(nc, psA, psM, ab, mp, dsp,
                  xt_sb, xs_sb, ws_sb, wh_sb, si_sb, od)
    if compile:
        nc.compile()
    return nc


def _body(nc, psA, psM, ab, mp, dsp,
          xt_sb, xs_sb, ws_sb, wh_sb, si_sb, od):
    # --- factor matmuls: A^T [32, 512], B^T [32, 512] --------------------
    a_t = ab.tile([R, 512], BF16, tag="a_t")
    pa = psA.tile([R, 512], F32, tag="pa")
    for k in range(8):
        nc.tensor.matmul(
            out=pa[:], lhsT=ws_sb[:, k, :], rhs=xt_sb[:, k, :],
            start=(k == 0), stop=(k == 7),
        )
    nc.vector.tensor_copy(out=a_t[:], in_=pa[:])
    b_t = ab.tile([R, 512], BF16, tag="b_t")
    pb = psA.tile([R, 512], F32, tag="pb")
    for k in range(8):
        nc.tensor.matmul(
            out=pb[:], lhsT=wh_sb[:, k, :], rhs=xs_sb[:, k, :],
            start=(k == 0), stop=(k == 7),
        )
    nc.scalar.copy(out=b_t[:], in_=pb[:])

    # --- M slice: [128, 2048] bf16 ---------------------------------------
    m_b = mp.tile([128, E], BF16, tag="m_b")
    for kb2 in range(2):
        pm = psM.tile([128, 1024], F32, tag="pm")
        for j in range(2):
            kb = 2 * kb2 + j
            nc.tensor.matmul(
                out=pm[:, j * 512:(j + 1) * 512],
                lhsT=a_t[:, kb * 128:(kb + 1) * 128], rhs=b_t[:],
                start=True, stop=True,
            )
        ceng = nc.vector.tensor_copy if kb2 == 0 else (
            lambda out, in_: nc.scalar.copy(out=out, in_=in_))
        ceng(out=m_b[:, kb2 * 1024:(kb2 + 1) * 1024], in_=pm[:])

    # --- local_scatter cascade ------------------------------------------
    ds = dsp.tile([128, ODW], BF16, tag="ds")
    for i in range(PASSES):
        data = m_b[:] if i == 0 else ds[:, DOFF[i - 1]:DOFF[i]]
        nc.gpsimd.local_scatter(
            out_ap=ds[:, DOFF[i]:DOFF[i + 1]], data_ap=data,
            idxs_ap=si_sb[:, IOFF[i]:IOFF[i + 1]],
            channels=128, num_elems=DSTS[i], num_idxs=SCANS[i],
        )
    cut1, cut2 = 1024, 2048
    nc.scalar.dma_start(
        out=bass.AP(tensor=od[:].tensor, offset=0,
                    ap=[[ODW, 128], [1, cut1]]),
        in_=ds[:, :cut1],
    )
    nc.sync.dma_start(
        out=bass.AP(tensor=od[:].tensor, offset=cut1,
                    ap=[[ODW, 128], [1, cut2 - cut1]]),
        in_=ds[:, cut1:cut2],
    )
    nc.scalar.dma_start(
        out=bass.AP(tensor=od[:].tensor, offset=cut2,
                    ap=[[ODW, 128], [1, ODW - cut2]]),
        in_=ds[:, cut2:],
    )


# ---------------------------------------------------------------------------
# Host-side routing
# ---------------------------------------------------------------------------

def _group_slots(keys):
    """Per-group running index for a sorted int array."""
    n = len(keys)
    if n == 0:
        return np.zeros(0, np.int64)
    first = np.r_[True, keys[1:] != keys[:-1]]
    starts = np.flatnonzero(first)
    counts = np.diff(np.r_[starts, n])
    return np.arange(n) - np.repeat(starts, counts)


def _route_quarter(s, h, n_sel):
    """Route one quarter's outputs through the scatter cascade.

    Returns (si_all [128, SIW] i16, (n_ids, od flat positions) for
    device-served users, fallback n_ids)."""
    p = (s & 127).astype(np.int64)
    o = ((((s >> 7) & 3) << 9) | (h & 511)).astype(np.int64)
    key = p * E + o
    order = np.argsort(key, kind="stable")
    ks = key[order]
    n_ord = n_sel[order]
    rank = _group_slots(ks)

    # element table (unique keys, key order)
    first = np.r_[True, ks[1:] != ks[:-1]]
    el_key = ks[first]
    el_cnt = np.diff(np.r_[np.flatnonzero(first), len(ks)])
    el_p = el_key // E
    el_o = el_key % E
    ne = len(el_key)
    u_el = np.cumsum(first) - 1           # user -> element index

    si_arr = np.full((128, SIW), -1, np.int16)
    el_slot = np.full((PASSES, ne), -1, np.int64)
    alive = np.ones(ne, bool)
    for k in range(PASSES):
        cand = alive & (el_cnt >= k + 1)
        idxs = np.flatnonzero(cand)
        slot = _group_slots(el_p[idxs])
        ovf = slot >= DSTS[k]
        if ovf.any():
            alive[idxs[ovf]] = False      # demote element's remaining users
            idxs, slot = idxs[~ovf], slot[~ovf]
        el_slot[k, idxs] = slot
        # device idx stream for pass k, indexed by data position j
        jpos = el_o[idxs] if k == 0 else el_slot[k - 1, idxs]
        si_arr[el_p[idxs], IOFF[k] + jpos] = slot.astype(np.int16)

    u_slot = np.where(rank < PASSES,
                      el_slot[np.minimum(rank, PASSES - 1), u_el], -1)
    okm = u_slot >= 0
    pos = (el_p[u_el[okm]] * ODW + np.asarray(DOFF)[rank[okm]]
           + u_slot[okm]).astype(np.int64)
    return si_arr, (n_ord[okm], pos), n_ord[~okm]


def prepare_in_maps(hidden_states, W_seq, W_hid, all_indices):
    x_bf = [np.ascontiguousarray(hidden_states[b].astype(ml_dtypes.bfloat16))
            for b in range(B)]
    ws_t = np.ascontiguousarray(W_seq.T.astype(ml_dtypes.bfloat16))
    wh_t = np.ascontiguousarray(W_hid.T.astype(ml_dtypes.bfloat16))

    s_idx = np.asarray(all_indices[:, 0], dtype=np.int64)
    h_idx = np.asarray(all_indices[:, 1], dtype=np.int64)
    qarr = 2 * (s_idx >> 9) + (h_idx >> 9)

    routes = []
    for q in range(4):
        n_sel = np.flatnonzero(qarr == q)
        routes.append(_route_quarter(s_idx[n_sel], h_idx[n_sel], n_sel))

    in_maps = []
    for c in range(NCORES):
        b, q = c // 4, c % 4
        si_arr, _, _ = routes[q]
        si_half, hj = q >> 1, q & 1
        in_maps.append({
            "xst": np.ascontiguousarray(x_bf[b][512 * si_half:512 * (si_half + 1), :]),
            "xs": np.ascontiguousarray(x_bf[b][:, 512 * hj:512 * (hj + 1)]),
            "wseq_t": ws_t,
            "whid_t": wh_t,
            "si_all": si_arr,
        })
    return in_maps, routes


def assemble(results, routes, hidden_states, W_seq, W_hid, all_indices):
    out = np.empty((B, N), dtype=np.float32)
    fb_cache = {}
    for c in range(NCORES):
        b, q = c // 4, c % 4
        _, (n_ids, pos), n_fb = routes[q]
        buf = np.asarray(results[c]["od"], np.float32).reshape(-1)
        out[b, n_ids] = buf[pos]
        if len(n_fb):
            # host fallback: elements that overflowed a dst cap or have
            # multiplicity > PASSES (never for uniform random indices)
            if b not in fb_cache:
                X = np.asarray(hidden_states[b], np.float32)
                A = X @ np.asarray(W_seq, np.float32).T        # [S, R]
                Bm = X.T @ np.asarray(W_hid, np.float32).T     # [H, R]
                fb_cache[b] = (A, Bm)
            A, Bm = fb_cache[b]
            si = np.asarray(all_indices[n_fb, 0], np.int64)
            hi = np.asarray(all_indices[n_fb, 1], np.int64)
            out[b, n_fb] = np.einsum("nr,nr->n", A[si], Bm[hi])
    return out.reshape(B, S, H)


# ---------------------------------------------------------------------------
# Runner (trace/compile SPMD executable once, reuse)
# ---------------------------------------------------------------------------

_nc_cache_by_reps = {}


def _get_nc(reps: int = 1):
    nc = _nc_cache_by_reps.get(reps)
    if nc is None:
        nc = _nc_cache_by_reps[reps] = _build(reps)
    return nc


class _Runner:
    """Trace/compile the SPMD executable once; reuse across calls."""

    def __init__(self, nc, donate=True):
        import jax
        from jax.experimental.shard_map import shard_map
        from jax.sharding import Mesh, PartitionSpec
        import concourse.bass2jax as b2j

        b2j.install_neuronx_cc_hook()
        self.nc = nc
        part_name = (nc.partition_id_tensor.name
                     if nc.partition_id_tensor else None)
        in_names, out_names, out_avals = [], [], []
        zero_outs = []
        for alloc in nc.m.functions[0].allocations:
            if not isinstance(alloc, mybir.MemoryLocationSet):
                continue
            name = alloc.memorylocations[0].name
            if alloc.kind == "ExternalInput":
                if name != part_name:
                    in_names.append(name)
            elif alloc.kind == "ExternalOutput":
                out_names.append(name)
                shape = tuple(alloc.tensor_shape)
                dtype = mybir.dt.np(alloc.dtype)
                out_avals.append(jax.core.ShapedArray(shape, dtype))
                zero_outs.append(np.zeros(shape, dtype))
        self.in_names = list(in_names)
        self.out_names = out_names
        self.zero_outs = zero_outs
        n_params = len(in_names)
        n_outs = len(out_names)
        all_in_names = in_names + out_names
        if part_name is not None:
            all_in_names = all_in_names + [part_name]
        donate_nums = (tuple(range(n_params, n_params + n_outs))
                       if donate else ())

        def _body_fn(*args):
            operands = list(args)
            if part_name is not None:
                operands.append(b2j.partition_id_tensor())
            outs = b2j._bass_exec_p.bind(
                *operands,
                out_avals=tuple(out_avals),
                in_names=tuple(all_in_names),
                out_names=tuple(out_names),
                lowering_input_output_aliases=(),
                sim_require_finite=True,
                sim_require_nnan=True,
                nc=nc,
            )
            return tuple(outs)

        devices = jax.devices()[:NCORES]
        mesh = Mesh(np.asarray(devices), ("core",))
        self.fn = jax.jit(
            shard_map(
                _body_fn, mesh=mesh,
                in_specs=(PartitionSpec("core"),) * (n_params + n_outs),
                out_specs=(PartitionSpec("core"),) * n_outs,
                check_rep=False,
            ),
            donate_argnums=donate_nums,
            keep_unused=True,
        )
        self.mesh = mesh

    def __call__(self, in_maps):
        concat_in = [
            np.concatenate([np.asarray(m[name]) for m in in_maps], axis=0)
            for name in self.in_names
        ]
        concat_zeros = [
            np.zeros((NCORES * z.shape[0], *z.shape[1:]), z.dtype)
            for z in self.zero_outs
        ]
        out_arrs = self.fn(*concat_in, *concat_zeros)
        return [
            {
                name: np.asarray(out_arrs[i]).reshape(
                    NCORES, *self.zero_outs[i].shape)[c]
                for i, name in enumerate(self.out_names)
            }
            for c in range(NCORES)
        ]


_runner_cache = {}


def _get_runner(reps: int = 1):
    r = _runner_cache.get(reps)
    if r is None:
        r = _runner_cache[reps] = _Runner(_get_nc(reps))
    return r


def kernel(hidden_states, W_seq, W_hid, all_indices):
    hidden_states = np.asarray(hidden_states)
    W_seq = np.asarray(W_seq)
    W_hid = np.asarray(W_hid)
    all_indices = np.asarray(all_indices)

    runner = _get_runner()
    in_maps, routes = prepare_in_maps(hidden_states, W_seq, W_hid, all_indices)
    results = runner(in_maps)
    return assemble(results, routes, hidden_states, W_seq, W_hid, all_indices)


# revision 6
# speedup vs baseline: 3.0819x; 3.0819x over previous
"""Trainium2 Bass kernel for nn_CPCircuitLayer (embedding_lookup), v4.

Math: out[b, n] = dot(A[b, idx_s[n]], Bm[b, idx_h[n]]) = M_b[idx_s[n], idx_h[n]]
where M_b = A_b @ Bm_b^T is a [S, H] table, A = X W_seq^T, Bm^T = W_hid X.

Sharding (8 cores, no collectives): core c handles batch b = c//4 and the
h-slice q = c%4 (columns [256q, 256q+256)). It builds only its M slice
[1024, 256] laid out as [128 partitions, 2048]: partition p = s%128,
offset o = (s//128)*256 + h%256. Each output n is routed to the core
owning its table element.

Per-core device pipeline:
  1. PE matmuls (bf16 in, f32 psum): A^T [32,1024], B^T [32,256], then
     M = A @ B^T -> sbuf bf16 [128, 2048].
  2. local_scatter cascade: pass k serves the rank-k user of every table
     element (host-prepped per-partition dst slots, in key order). Pass 0
     scans the table; pass k>=1 scans pass (k-1)'s dst buffer, which is
     exactly the compacted list of elements with >= k users. Scan lengths
     shrink geometrically, so 12 passes (multiplicity <= 12) cost barely
     more than 3. The packed dst buffer IS the output: one DMA to DRAM
     (split across queues), host unpermutes.
  3. Outputs whose element overflowed a dst cap or has multiplicity > 12
     (never happens for uniform random indices) are computed on host.
"""

import numpy as np
import ml_dtypes
from contextlib import ExitStack

import concourse.bass as bass
import concourse.mybir as mybir
import concourse.tile as tile
from concourse import bacc

B, S, H, R = 2, 1024, 1024, 32
N = S * H
NCORES = 8
HQ = H // 4           # h-columns per core
E = 2048              # table elements per partition (1024*256/128)
DSTS = [1440, 672, 240, 72, 24, 16, 8, 4, 4]  # per-pass dst caps
PASSES = len(DSTS)
SCANS = [E] + DSTS[:-1]                # pass k scans pass k-1's dst
DOFF = np.cumsum([0] + DSTS).tolist()  # dst offsets in packed output
IOFF = np.cumsum([0] + SCANS).tolist()  # idx offsets in packed si input
ODW = DOFF[-1]        # packed output width (2832)
SIW = IOFF[-1]        # packed int16 input width (4872)

F32 = mybir.dt.float32
BF16 = mybir.dt.bfloat16
I16 = mybir.dt.int16


def _build(reps: int = 1, compile: bool = True):
    nc = bacc.Bacc()
    xst = nc.declare_dram_parameter("xst", [512, H], BF16, False)
    xs = nc.declare_dram_parameter("xs", [S, 512], BF16, False)
    wseq_t = nc.declare_dram_parameter("wseq_t", [H, R], BF16, False)
    whid_t = nc.declare_dram_parameter("whid_t", [S, R], BF16, False)
    si_all = nc.declare_dram_parameter("si_all", [128, SIW], I16, False)
    od = nc.declare_dram_parameter("od", [128, ODW], BF16, True)

    with tile.TileContext(nc) as tc, ExitStack() as ctx:
        base = ctx.enter_context(tc.tile_pool(name="base", bufs=1))
        psA = ctx.enter_context(tc.tile_pool(name="psA", bufs=2, space="PSUM"))
        psM = ctx.enter_context(tc.tile_pool(name="psM", bufs=2, space="PSUM"))
        ab = ctx.enter_context(tc.tile_pool(name="ab", bufs=2))
        mp = ctx.enter_context(tc.tile_pool(name="mp", bufs=2))
        dsp = ctx.enter_context(tc.tile_pool(name="dsp", bufs=2))

        # --- one-time loads ----------------------------------------------
        xt_sb = base.tile([128, 8, 512], BF16)    # X^T[h,s-half]
        xs_sb = base.tile([128, 8, 512], BF16)    # X[:, h-half]
        ws_sb = base.tile([128, 8, R], BF16)      # W_seq^T rows (h-major)
        wh_sb = base.tile([128, 8, R], BF16)      # W_hid^T rows (s-major)
        si_sb = base.tile([128, SIW], I16)        # cascade idx streams

        for k in range(8):
            nc.sync.dma_start_transpose(
                out=xt_sb[:, k, :], in_=xst[:, 128 * k:128 * (k + 1)]
            )
        nc.sync.dma_start(
            out=xs_sb[:],
            in_=bass.AP(tensor=xs[:].tensor, offset=0,
                        ap=[[512, 128], [128 * 512, 8], [1, 512]]),
        )
        nc.sync.dma_start(
            out=ws_sb[:],
            in_=bass.AP(tensor=wseq_t[:].tensor, offset=0,
                        ap=[[R, 128], [128 * R, 8], [1, R]]),
        )
        nc.sync.dma_start(
            out=wh_sb[:],
            in_=bass.AP(tensor=whid_t[:].tensor, offset=0,
                        ap=[[R, 128], [128 * R, 8], [1, R]]),
        )
        nc.sync.dma_start(out=si_sb[:], in_=si_all[:])

        for _ in range(reps):
            _body(nc, psA, psM, ab, mp, dsp,
                  xt_sb, xs_sb, ws_sb, wh_sb, si_sb, od)
    if compile:
        nc.compile()
    return nc


def _body(nc, psA, psM, ab, mp, dsp,
          xt_sb, xs_sb, ws_sb, wh_sb, si_sb, od):
    # --- factor matmuls: A^T [32, 512], B^T [32, 512] --------------------
    a_t = ab.tile([R, 512], BF16, tag="a_t")
    pa = psA.tile([R, 512], F32, tag="pa")
    for k in range(8):
        nc.tensor.matmul(
            out=pa[:], lhsT=ws_sb[:, k, :], rhs=xt_sb[:, k, :],
            start=(k == 0), stop=(k == 7),
        )
    nc.vector.tensor_copy(out=a_t[:], in_=pa[:])
    b_t = ab.tile([R, 512], BF16, tag="b_t")
    pb = psA.tile([R, 512], F32, tag="pb")
    for k in range(8):
        nc.tensor.matmul(
            out=pb[:], lhsT=wh_sb[:, k, :], rhs=xs_sb[:, k, :],
            start=(k == 0), stop=(k == 7),
        )
    nc.scalar.copy(out=b_t[:], in_=pb[:])

    # --- M slice: [128, 2048] bf16 ---------------------------------------
    m_b = mp.tile([128, E], BF16, tag="m_b")
    for kb2 in range(2):
        pm = psM.tile([128, 1024], F32, tag="pm")
        for j in range(2):
            kb = 2 * kb2 + j
            nc.tensor.matmul(
                out=pm[:, j * 512:(j + 1) * 512],
                lhsT=a_t[:, kb * 128:(kb + 1) * 128], rhs=b_t[:],
                start=True, stop=True,
            )
        ceng = nc.vector.tensor_copy if kb2 == 0 else (
            lambda out, in_: nc.scalar.copy(out=out, in_=in_))
        ceng(out=m_b[:, kb2 * 1024:(kb2 + 1) * 1024], in_=pm[:])

    # --- local_scatter cascade ------------------------------------------
    ds = dsp.tile([128, ODW], BF16, tag="ds")
    for i in range(PASSES):
        data = m_b[:] if i == 0 else ds[:, DOFF[i - 1]:DOFF[i]]
        nc.gpsimd.local_scatter(
            out_ap=ds[:, DOFF[i]:DOFF[i + 1]], data_ap=data,
            idxs_ap=si_sb[:, IOFF[i]:IOFF[i + 1]],
            channels=128, num_elems=DSTS[i], num_idxs=SCANS[i],
        )
    cut1, cut2 = 1024, 2048
    nc.scalar.dma_start(
        out=bass.AP(tensor=od[:].tensor, offset=0,
                    ap=[[ODW, 128], [1, cut1]]),
        in_=ds[:, :cut1],
    )
    nc.sync.dma_start(
        out=bass.AP(tensor=od[:].tensor, offset=cut1,
                    ap=[[ODW, 128], [1, cut2 - cut1]]),
        in_=ds[:, cut1:cut2],
    )
    nc.sync.dma_start(
        out=bass.AP(tensor=od[:].tensor, offset=cut2,
                    ap=[[ODW, 128], [1, ODW - cut2]]),
        in_=ds[:, cut2:],
    )


# ---------------------------------------------------------------------------
# Host-side routing
# ---------------------------------------------------------------------------

def _group_slots(keys):
    """Per-group running index for a sorted int array."""
    n = len(keys)
    if n == 0:
        return np.zeros(0, np.int64)
    first = np.r_[True, keys[1:] != keys[:-1]]
    starts = np.flatnonzero(first)
    counts = np.diff(np.r_[starts, n])
    return np.arange(n) - np.repeat(starts, counts)


def _route_quarter(s, h, n_sel):
    """Route one quarter's outputs through the scatter cascade.

    Returns (si_all [128, SIW] i16, (n_ids, od flat positions) for
    device-served users, fallback n_ids)."""
    p = (s & 127).astype(np.int64)
    o = ((((s >> 7) & 3) << 9) | (h & 511)).astype(np.int64)
    key = p * E + o
    order = np.argsort(key, kind="stable")
    ks = key[order]
    n_ord = n_sel[order]
    rank = _group_slots(ks)

    # element table (unique keys, key order)
    first = np.r_[True, ks[1:] != ks[:-1]]
    el_key = ks[first]
    el_cnt = np.diff(np.r_[np.flatnonzero(first), len(ks)])
    el_p = el_key // E
    el_o = el_key % E
    ne = len(el_key)
    u_el = np.cumsum(first) - 1           # user -> element index

    si_arr = np.full((128, SIW), -1, np.int16)
    el_slot = np.full((PASSES, ne), -1, np.int64)
    alive = np.ones(ne, bool)
    for k in range(PASSES):
        cand = alive & (el_cnt >= k + 1)
        idxs = np.flatnonzero(cand)
        slot = _group_slots(el_p[idxs])
        ovf = slot >= DSTS[k]
        if ovf.any():
            alive[idxs[ovf]] = False      # demote element's remaining users
            idxs, slot = idxs[~ovf], slot[~ovf]
        el_slot[k, idxs] = slot
        # device idx stream for pass k, indexed by data position j
        jpos = el_o[idxs] if k == 0 else el_slot[k - 1, idxs]
        si_arr[el_p[idxs], IOFF[k] + jpos] = slot.astype(np.int16)

    u_slot = np.where(rank < PASSES,
                      el_slot[np.minimum(rank, PASSES - 1), u_el], -1)
    okm = u_slot >= 0
    pos = (el_p[u_el[okm]] * ODW + np.asarray(DOFF)[rank[okm]]
           + u_slot[okm]).astype(np.int64)
    return si_arr, (n_ord[okm], pos), n_ord[~okm]


def prepare_in_maps(hidden_states, W_seq, W_hid, all_indices):
    x_bf = [np.ascontiguousarray(hidden_states[b].astype(ml_dtypes.bfloat16))
            for b in range(B)]
    ws_t = np.ascontiguousarray(W_seq.T.astype(ml_dtypes.bfloat16))
    wh_t = np.ascontiguousarray(W_hid.T.astype(ml_dtypes.bfloat16))

    s_idx = np.asarray(all_indices[:, 0], dtype=np.int64)
    h_idx = np.asarray(all_indices[:, 1], dtype=np.int64)
    qarr = 2 * (s_idx >> 9) + (h_idx >> 9)

    routes = []
    for q in range(4):
        n_sel = np.flatnonzero(qarr == q)
        routes.append(_route_quarter(s_idx[n_sel], h_idx[n_sel], n_sel))

    in_maps = []
    for c in range(NCORES):
        b, q = c // 4, c % 4
        si_arr, _, _ = routes[q]
        si_half, hj = q >> 1, q & 1
        in_maps.append({
            "xst": np.ascontiguousarray(x_bf[b][512 * si_half:512 * (si_half + 1), :]),
            "xs": np.ascontiguousarray(x_bf[b][:, 512 * hj:512 * (hj + 1)]),
            "wseq_t": ws_t,
            "whid_t": wh_t,
            "si_all": si_arr,
        })
    return in_maps, routes


def assemble(results, routes, hidden_states, W_seq, W_hid, all_indices):
    out = np.empty((B, N), dtype=np.float32)
    fb_cache = {}
    for c in range(NCORES):
        b, q = c // 4, c % 4
        _, (n_ids, pos), n_fb = routes[q]
        buf = np.asarray(results[c]["od"], np.float32).reshape(-1)
        out[b, n_ids] = buf[pos]
        if len(n_fb):
            # host fallback: elements that overflowed a dst cap or have
            # multiplicity > PASSES (never for uniform random indices)
            if b not in fb_cache:
                X = np.asarray(hidden_states[b], np.float32)
                A = X @ np.asarray(W_seq, np.float32).T        # [S, R]
                Bm = X.T @ np.asarray(W_hid, np.float32).T     # [H, R]
                fb_cache[b] = (A, Bm)
            A, Bm = fb_cache[b]
            si = np.asarray(all_indices[n_fb, 0], np.int64)
            hi = np.asarray(all_indices[n_fb, 1], np.int64)
            out[b, n_fb] = np.einsum("nr,nr->n", A[si], Bm[hi])
    return out.reshape(B, S, H)


# ---------------------------------------------------------------------------
# Runner (trace/compile SPMD executable once, reuse)
# ---------------------------------------------------------------------------

_nc_cache_by_reps = {}


def _get_nc(reps: int = 1):
    nc = _nc_cache_by_reps.get(reps)
    if nc is None:
        nc = _nc_cache_by_reps[reps] = _build(reps)
    return nc


class _Runner:
    """Trace/compile the SPMD executable once; reuse across calls."""

    def __init__(self, nc, donate=True):
        import jax
        from jax.experimental.shard_map import shard_map
        from jax.sharding import Mesh, PartitionSpec
        import concourse.bass2jax as b2j

        b2j.install_neuronx_cc_hook()
        self.nc = nc
        part_name = (nc.partition_id_tensor.name
                     if nc.partition_id_tensor else None)
        in_names, out_names, out_avals = [], [], []
        zero_outs = []
        for alloc in nc.m.functions[0].allocations:
            if not isinstance(alloc, mybir.MemoryLocationSet):
                continue
            name = alloc.memorylocations[0].name
            if alloc.kind == "ExternalInput":
                if name != part_name:
                    in_names.append(name)
            elif alloc.kind == "ExternalOutput":
                out_names.append(name)
                shape = tuple(alloc.tensor_shape)
                dtype = mybir.dt.np(alloc.dtype)
                out_avals.append(jax.core.ShapedArray(shape, dtype))
                zero_outs.append(np.zeros(shape, dtype))
        self.in_names = list(in_names)
        self.out_names = out_names
        self.zero_outs = zero_outs
        n_params = len(in_names)
        n_outs = len(out_names)
        all_in_names = in_names + out_names
        if part_name is not None:
            all_in_names = all_in_names + [part_name]
        donate_nums = (tuple(range(n_params, n_params + n_outs))
                       if donate else ())

        def _body_fn(*args):
            operands = list(args)
            if part_name is not None:
                operands.append(b2j.partition_id_tensor())
            outs = b2j._bass_exec_p.bind(
                *operands,
                out_avals=tuple(out_avals),
                in_names=tuple(all_in_names),
                out_names=tuple(out_names),
                lowering_input_output_aliases=(),
                sim_require_finite=True,
                sim_require_nnan=True,
                nc=nc,
            )
            return tuple(outs)

        devices = jax.devices()[:NCORES]
        mesh = Mesh(np.asarray(devices), ("core",))
        self.fn = jax.jit(
            shard_map(
                _body_fn, mesh=mesh,
                in_specs=(PartitionSpec("core"),) * (n_params + n_outs),
                out_specs=(PartitionSpec("core"),) * n_outs,
                check_rep=False,
            ),
            donate_argnums=donate_nums,
            keep_unused=True,
        )
        self.mesh = mesh

    def __call__(self, in_maps):
        concat_in = [
            np.concatenate([np.asarray(m[name]) for m in in_maps], axis=0)
            for name in self.in_names
        ]
        concat_zeros = [
            np.zeros((NCORES * z.shape[0], *z.shape[1:]), z.dtype)
            for z in self.zero_outs
        ]
        out_arrs = self.fn(*concat_in, *concat_zeros)
        return [
            {
                name: np.asarray(out_arrs[i]).reshape(
                    NCORES, *self.zero_outs[i].shape)[c]
                for i, name in enumerate(self.out_names)
            }
            for c in range(NCORES)
        ]


_runner_cache = {}


def _get_runner(reps: int = 1):
    r = _runner_cache.get(reps)
    if r is None:
        r = _runner_cache[reps] = _Runner(_get_nc(reps))
    return r


def kernel(hidden_states, W_seq, W_hid, all_indices):
    hidden_states = np.asarray(hidden_states)
    W_seq = np.asarray(W_seq)
    W_hid = np.asarray(W_hid)
    all_indices = np.asarray(all_indices)

    runner = _get_runner()
    in_maps, routes = prepare_in_maps(hidden_states, W_seq, W_hid, all_indices)
    results = runner(in_maps)
    return assemble(results, routes, hidden_states, W_seq, W_hid, all_indices)
